# revision 8
# baseline (speedup 1.0000x reference)
"""Jaccard index (IoU) kernel for Trainium2, 8 NeuronCores.

Problem: preds [8, 21, 512, 512] f32 uniform(0,1), target [8, 21, 512, 512]
f32 in {0.0, 1.0}. Per class c:
    pred_mask   = preds >= 0.5
    target_mask = target == 1.0
    inter[c] = sum(pred_mask & target_mask), over batch+spatial
    union[c] = sum(pred_mask) + sum(target_mask) - inter[c]
    iou[c]   = nan if union == 0 else inter / max(union, 1)

Strategy (data-parallel over batch, one batch element per core):
  Per core, per class, load p,t as [128, 2048] f32 tiles and compute three
  per-partition row-sum accumulators with fused reduce ops (accum_out):
    ACT : t2 = 1.5 - t            -> A_t[:,c] = rowsum(1.5 - t)  (recovers sum(t))
    DVE : TTR (p is_ge t2)        -> A_i[:,c] = rowsum(p + t >= 1.5) = inter rows
    DVE : TS  (p is_ge 0.5)       -> A_p[:,c] = rowsum(pred_mask)
  (p >= 1.5 - t  <=>  p + t >= 1.5  <=>  pred_mask AND t == 1, exactly, since
   t is exactly 0.0 or 1.0.)
  Each core DMAs the three [128, 21] accumulators out; the host sums the
  8*128 partials per class in float64 (exact: all values are multiples of
  0.5 below 2^23) and does the final divide / nan handling.
"""

import os
import sys

import numpy as np

for _p in ("/root/.axon_site/_ro/trn_rl_repo", "/opt/trn_rl_repo"):
    if os.path.isdir(_p) and _p not in sys.path:
        sys.path.insert(0, _p)

import concourse.bacc as bacc
import concourse.tile as tile
from concourse import mybir
from concourse.bass_utils import run_bass_kernel_spmd

B, C, HH, WW = 8, 21, 512, 512
P, F = 128, 2048  # per-class tile: 512*512 == 128*2048
N_CORES = 8

_nc_cache = None


def build_nc(io_bufs=4, aux_bufs=3, group=1):
    f32 = mybir.dt.float32
    nc = bacc.Bacc(None, target_bir_lowering=False)
    preds = nc.dram_tensor("preds", [C, P, F], f32, kind="ExternalInput")
    target = nc.dram_tensor("target", [C, P, F], f32, kind="ExternalInput")
    out = nc.dram_tensor("partials", [3, P, C], f32, kind="ExternalOutput")

    with tile.TileContext(nc) as tc:
        with tc.tile_pool(name="io", bufs=io_bufs) as io_pool, \
             tc.tile_pool(name="aux", bufs=aux_bufs) as aux_pool, \
             tc.tile_pool(name="acc", bufs=1) as acc_pool:
            A_p = acc_pool.tile([P, C], f32, tag="A_p")
            A_i = acc_pool.tile([P, C], f32, tag="A_i")
            A_t = acc_pool.tile([P, C], f32, tag="A_t")
            for c0 in range(0, C, group):
                g = min(group, C - c0)
                # One DMA per tensor per group of g classes: [128, g*2048].
                p_t = io_pool.tile([P, group, F], f32, tag="p")
                t_t = io_pool.tile([P, group, F], f32, tag="t")
                src_p = preds[c0 : c0 + g].rearrange("g p f -> p g f")
                src_t = target[c0 : c0 + g].rearrange("g p f -> p g f")
                nc.sync.dma_start(out=p_t[:, :g], in_=src_p)
                nc.sync.dma_start(out=t_t[:, :g], in_=src_t)
                for j in range(g):
                    c = c0 + j
                    pv = p_t[:, j]
                    tv = t_t[:, j]
                    mask_p = aux_pool.tile([P, F], f32, tag="mask_p")
                    jt = aux_pool.tile([P, F], f32, tag="jt")
                    # ACT: copy t; A_t[:,c] = rowsum(t) = target count.
                    nc.scalar.activation(
                        out=jt,
                        in_=tv,
                        func=mybir.ActivationFunctionType.Copy,
                        bias=0.0,
                        scale=1.0,
                        accum_out=A_t[:, c : c + 1],
                    )
                    # DVE TS: mask_p = (p >= 0.5); A_p[:,c] = rowsum.
                    nc.vector.tensor_scalar(
                        out=mask_p,
                        in0=pv,
                        scalar1=0.5,
                        scalar2=None,
                        op0=mybir.AluOpType.is_ge,
                        op1=mybir.AluOpType.add,
                        accum_out=A_p[:, c : c + 1],
                    )
                    # DVE TT: m_i = mask_p AND t; write over pv (dead).
                    nc.vector.tensor_tensor(
                        out=pv,
                        in0=mask_p,
                        in1=tv,
                        op=mybir.AluOpType.logical_and,
                    )
                    # ACT: copy m_i; A_i[:,c] = rowsum. Write over tv (dead).
                    nc.scalar.activation(
                        out=tv,
                        in_=pv,
                        func=mybir.ActivationFunctionType.Copy,
                        bias=0.0,
                        scale=1.0,
                        accum_out=A_i[:, c : c + 1],
                    )
            nc.sync.dma_start(out=out[0], in_=A_p)
            nc.sync.dma_start(out=out[1], in_=A_i)
            nc.sync.dma_start(out=out[2], in_=A_t)
    nc.finalize()
    return nc


def _get_nc():
    global _nc_cache
    if _nc_cache is None:
        _nc_cache = build_nc()
    return _nc_cache


def _run(preds, target, **spmd_kwargs):
    nc = _get_nc()
    preds = np.asarray(preds, dtype=np.float32)
    target = np.asarray(target, dtype=np.float32)
    in_maps = [
        {
            "preds": np.ascontiguousarray(preds[i]).reshape(C, P, F),
            "target": np.ascontiguousarray(target[i]).reshape(C, P, F),
        }
        for i in range(N_CORES)
    ]
    res = run_bass_kernel_spmd(nc, in_maps, core_ids=list(range(N_CORES)), **spmd_kwargs)
    parts = np.stack([r["partials"] for r in res.results], 0).astype(np.float64)
    sums = parts.sum(axis=(0, 2))  # [3, C]
    S_p, S_i, S_t = sums[0], sums[1], sums[2]
    union = S_p + S_t - S_i
    with np.errstate(invalid="ignore", divide="ignore"):
        iou = np.where(union == 0.0, np.nan, S_i / np.maximum(union, 1.0))
    return iou.astype(np.float32), res


def kernel(preds, target):
    iou, _ = _run(preds, target)
    return iou


# revision 9
# speedup vs baseline: 1.0066x; 1.0066x over previous
"""Jaccard index (IoU) kernel for Trainium2, 8 NeuronCores.

Problem: preds [8, 21, 512, 512] f32 uniform(0,1), target [8, 21, 512, 512]
f32 in {0.0, 1.0}. Per class c:
    pred_mask   = preds >= 0.5
    target_mask = target == 1.0
    inter[c] = sum(pred_mask & target_mask), over batch+spatial
    union[c] = sum(pred_mask) + sum(target_mask) - inter[c]
    iou[c]   = nan if union == 0 else inter / max(union, 1)

Strategy (data-parallel over batch, one batch element per core):
  Per core, per class, load p,t as [128, 2048] f32 tiles and compute three
  per-partition row-sum accumulators with fused reduce ops (accum_out):
    ACT : t2 = 1.5 - t            -> A_t[:,c] = rowsum(1.5 - t)  (recovers sum(t))
    DVE : TTR (p is_ge t2)        -> A_i[:,c] = rowsum(p + t >= 1.5) = inter rows
    DVE : TS  (p is_ge 0.5)       -> A_p[:,c] = rowsum(pred_mask)
  (p >= 1.5 - t  <=>  p + t >= 1.5  <=>  pred_mask AND t == 1, exactly, since
   t is exactly 0.0 or 1.0.)
  Each core DMAs the three [128, 21] accumulators out; the host sums the
  8*128 partials per class in float64 (exact: all values are multiples of
  0.5 below 2^23) and does the final divide / nan handling.
"""

import os
import sys

import numpy as np

for _p in ("/root/.axon_site/_ro/trn_rl_repo", "/opt/trn_rl_repo"):
    if os.path.isdir(_p) and _p not in sys.path:
        sys.path.insert(0, _p)

import concourse.bacc as bacc
import concourse.tile as tile
from concourse import mybir
from concourse.bass_utils import run_bass_kernel_spmd

B, C, HH, WW = 8, 21, 512, 512
P, F = 128, 2048  # per-class tile: 512*512 == 128*2048
N_CORES = 8

_nc_cache = None


def build_nc(io_bufs=4, aux_bufs=3, group=1):
    f32 = mybir.dt.float32
    nc = bacc.Bacc(None, target_bir_lowering=False)
    preds = nc.dram_tensor("preds", [C, P, F], f32, kind="ExternalInput")
    target = nc.dram_tensor("target", [C, P, F], f32, kind="ExternalInput")
    out = nc.dram_tensor("partials", [3, P, C], f32, kind="ExternalOutput")

    with tile.TileContext(nc) as tc:
        with tc.tile_pool(name="io", bufs=io_bufs) as io_pool, \
             tc.tile_pool(name="aux", bufs=aux_bufs) as aux_pool, \
             tc.tile_pool(name="acc", bufs=1) as acc_pool:
            A_p = acc_pool.tile([P, C], f32, tag="A_p")
            A_i = acc_pool.tile([P, C], f32, tag="A_i")
            A_t = acc_pool.tile([P, C], f32, tag="A_t")
            for c0 in range(0, C, group):
                g = min(group, C - c0)
                # One DMA per tensor per group of g classes: [128, g*2048].
                p_t = io_pool.tile([P, group, F], f32, tag="p")
                t_t = io_pool.tile([P, group, F], f32, tag="t")
                src_p = preds[c0 : c0 + g].rearrange("g p f -> p g f")
                src_t = target[c0 : c0 + g].rearrange("g p f -> p g f")
                nc.sync.dma_start(out=p_t[:, :g], in_=src_p)
                nc.sync.dma_start(out=t_t[:, :g], in_=src_t)
                for j in range(g):
                    c = c0 + j
                    pv = p_t[:, j]
                    tv = t_t[:, j]
                    mask_p = aux_pool.tile([P, F], f32, tag="mask_p")
                    jt = aux_pool.tile([P, F], f32, tag="jt")
                    # ACT: copy t; A_t[:,c] = rowsum(t) = target count.
                    nc.scalar.activation(
                        out=jt,
                        in_=tv,
                        func=mybir.ActivationFunctionType.Copy,
                        bias=0.0,
                        scale=1.0,
                        accum_out=A_t[:, c : c + 1],
                    )
                    # DVE TS: mask_p = (p >= 0.5); A_p[:,c] = rowsum.
                    nc.vector.tensor_scalar(
                        out=mask_p,
                        in0=pv,
                        scalar1=0.5,
                        scalar2=None,
                        op0=mybir.AluOpType.is_ge,
                        op1=mybir.AluOpType.add,
                        accum_out=A_p[:, c : c + 1],
                    )
                    # DVE TT: m_i = mask_p AND t; write over pv (dead).
                    nc.vector.tensor_tensor(
                        out=pv,
                        in0=mask_p,
                        in1=tv,
                        op=mybir.AluOpType.logical_and,
                    )
                    # DVE TS: m_i >= 0.5 is a copy of m_i (exactly 0/1);
                    # A_i[:,c] = rowsum. 2x perf mode; shorter post-DMA tail
                    # than a second ACT copy. Write over tv (dead).
                    nc.vector.tensor_scalar(
                        out=tv,
                        in0=pv,
                        scalar1=0.5,
                        scalar2=None,
                        op0=mybir.AluOpType.is_ge,
                        op1=mybir.AluOpType.add,
                        accum_out=A_i[:, c : c + 1],
                    )
            nc.sync.dma_start(out=out[0], in_=A_p)
            nc.sync.dma_start(out=out[1], in_=A_i)
            nc.sync.dma_start(out=out[2], in_=A_t)
    nc.finalize()
    return nc


def _get_nc():
    global _nc_cache
    if _nc_cache is None:
        _nc_cache = build_nc()
    return _nc_cache


def _run(preds, target, **spmd_kwargs):
    nc = _get_nc()
    preds = np.asarray(preds, dtype=np.float32)
    target = np.asarray(target, dtype=np.float32)
    in_maps = [
        {
            "preds": np.ascontiguousarray(preds[i]).reshape(C, P, F),
            "target": np.ascontiguousarray(target[i]).reshape(C, P, F),
        }
        for i in range(N_CORES)
    ]
    res = run_bass_kernel_spmd(nc, in_maps, core_ids=list(range(N_CORES)), **spmd_kwargs)
    parts = np.stack([r["partials"] for r in res.results], 0).astype(np.float64)
    sums = parts.sum(axis=(0, 2))  # [3, C]
    S_p, S_i, S_t = sums[0], sums[1], sums[2]
    union = S_p + S_t - S_i
    with np.errstate(invalid="ignore", divide="ignore"):
        iou = np.where(union == 0.0, np.nan, S_i / np.maximum(union, 1.0))
    return iou.astype(np.float32), res


def kernel(preds, target):
    iou, _ = _run(preds, target)
    return iou


# revision 11
# speedup vs baseline: 1.0196x; 1.0129x over previous
"""Jaccard index (IoU) kernel for Trainium2, 8 NeuronCores.

Problem: preds [8, 21, 512, 512] f32 uniform(0,1), target [8, 21, 512, 512]
f32 in {0.0, 1.0}. Per class c:
    pred_mask   = preds >= 0.5
    target_mask = target == 1.0
    inter[c] = sum(pred_mask & target_mask), over batch+spatial
    union[c] = sum(pred_mask) + sum(target_mask) - inter[c]
    iou[c]   = nan if union == 0 else inter / max(union, 1)

Strategy (data-parallel over batch, one batch element per core):
  Per core, per class, load p,t as [128, 2048] f32 tiles and compute three
  per-partition row-sum accumulators with fused reduce ops (accum_out):
    ACT : t2 = 1.5 - t            -> A_t[:,c] = rowsum(1.5 - t)  (recovers sum(t))
    DVE : TTR (p is_ge t2)        -> A_i[:,c] = rowsum(p + t >= 1.5) = inter rows
    DVE : TS  (p is_ge 0.5)       -> A_p[:,c] = rowsum(pred_mask)
  (p >= 1.5 - t  <=>  p + t >= 1.5  <=>  pred_mask AND t == 1, exactly, since
   t is exactly 0.0 or 1.0.)
  Each core DMAs the three [128, 21] accumulators out; the host sums the
  8*128 partials per class in float64 (exact: all values are multiples of
  0.5 below 2^23) and does the final divide / nan handling.
"""

import os
import sys

import numpy as np

for _p in ("/root/.axon_site/_ro/trn_rl_repo", "/opt/trn_rl_repo"):
    if os.path.isdir(_p) and _p not in sys.path:
        sys.path.insert(0, _p)

import concourse.bacc as bacc
import concourse.tile as tile
from concourse import mybir
from concourse.bass_utils import run_bass_kernel_spmd

B, C, HH, WW = 8, 21, 512, 512
P, F = 128, 2048  # per-class tile: 512*512 == 128*2048
N_CORES = 8

_nc_cache = None


NSPLIT = 2  # halves per class: compute starts after 512 KiB, tail chain halves
NCOL = C * NSPLIT


def build_nc(io_bufs=4, aux_bufs=3):
    f32 = mybir.dt.float32
    H = F // NSPLIT
    nc = bacc.Bacc(None, target_bir_lowering=False)
    preds = nc.dram_tensor("preds", [C, P, F], f32, kind="ExternalInput")
    target = nc.dram_tensor("target", [C, P, F], f32, kind="ExternalInput")
    out = nc.dram_tensor("partials", [3, P, NCOL], f32, kind="ExternalOutput")

    with tile.TileContext(nc) as tc:
        with tc.tile_pool(name="io", bufs=io_bufs) as io_pool, \
             tc.tile_pool(name="aux", bufs=aux_bufs) as aux_pool, \
             tc.tile_pool(name="acc", bufs=1) as acc_pool:
            A_p = acc_pool.tile([P, NCOL], f32, tag="A_p")
            A_i = acc_pool.tile([P, NCOL], f32, tag="A_i")
            A_t = acc_pool.tile([P, NCOL], f32, tag="A_t")
            for c in range(C):
                for s in range(NSPLIT):
                    k = c * NSPLIT + s
                    p_t = io_pool.tile([P, H], f32, tag="p")
                    t_t = io_pool.tile([P, H], f32, tag="t")
                    nc.sync.dma_start(out=p_t, in_=preds[c, :, s * H : (s + 1) * H])
                    nc.sync.dma_start(out=t_t, in_=target[c, :, s * H : (s + 1) * H])
                    mask_p = aux_pool.tile([P, H], f32, tag="mask_p")
                    jt = aux_pool.tile([P, H], f32, tag="jt")
                    # ACT: copy t; A_t[:,k] = rowsum(t) = target count.
                    nc.scalar.activation(
                        out=jt,
                        in_=t_t,
                        func=mybir.ActivationFunctionType.Copy,
                        bias=0.0,
                        scale=1.0,
                        accum_out=A_t[:, k : k + 1],
                    )
                    # DVE TS: mask_p = (p >= 0.5); A_p[:,k] = rowsum.
                    nc.vector.tensor_scalar(
                        out=mask_p,
                        in0=p_t,
                        scalar1=0.5,
                        scalar2=None,
                        op0=mybir.AluOpType.is_ge,
                        op1=mybir.AluOpType.add,
                        accum_out=A_p[:, k : k + 1],
                    )
                    # DVE TT: m_i = mask_p AND t; write over p_t (dead).
                    nc.vector.tensor_tensor(
                        out=p_t,
                        in0=mask_p,
                        in1=t_t,
                        op=mybir.AluOpType.logical_and,
                    )
                    # DVE TS: m_i >= 0.5 is a copy of m_i (exactly 0/1);
                    # A_i[:,k] = rowsum. 2x perf mode. Write over t_t (dead).
                    nc.vector.tensor_scalar(
                        out=t_t,
                        in0=p_t,
                        scalar1=0.5,
                        scalar2=None,
                        op0=mybir.AluOpType.is_ge,
                        op1=mybir.AluOpType.add,
                        accum_out=A_i[:, k : k + 1],
                    )
            nc.sync.dma_start(out=out[0], in_=A_p)
            nc.sync.dma_start(out=out[1], in_=A_i)
            nc.sync.dma_start(out=out[2], in_=A_t)
    nc.finalize()
    return nc


def _get_nc():
    global _nc_cache
    if _nc_cache is None:
        _nc_cache = build_nc()
    return _nc_cache


def _run(preds, target, **spmd_kwargs):
    nc = _get_nc()
    preds = np.asarray(preds, dtype=np.float32)
    target = np.asarray(target, dtype=np.float32)
    in_maps = [
        {
            "preds": np.ascontiguousarray(preds[i]).reshape(C, P, F),
            "target": np.ascontiguousarray(target[i]).reshape(C, P, F),
        }
        for i in range(N_CORES)
    ]
    res = run_bass_kernel_spmd(nc, in_maps, core_ids=list(range(N_CORES)), **spmd_kwargs)
    parts = np.stack([r["partials"] for r in res.results], 0).astype(np.float64)
    sums = parts.sum(axis=(0, 2))  # [3, C*NSPLIT]
    sums = sums.reshape(3, C, NSPLIT).sum(axis=2)  # [3, C]
    S_p, S_i, S_t = sums[0], sums[1], sums[2]
    union = S_p + S_t - S_i
    with np.errstate(invalid="ignore", divide="ignore"):
        iou = np.where(union == 0.0, np.nan, S_i / np.maximum(union, 1.0))
    return iou.astype(np.float32), res


def kernel(preds, target):
    iou, _ = _run(preds, target)
    return iou
